# revision 1
# baseline (speedup 1.0000x reference)
"""Trainium2 Bass kernel for a dense transformer block (B=2, T=2048, C=1024,
H=16, D=64, FF=4096), SPMD on 8 NeuronCores.

Sharding: pure data-parallel over tokens, zero collectives.
  core cid -> batch b = cid // 4, rank r = cid % 4.
  Each batch's 2048 tokens split into 8 causal chunks of 256; rank r owns
  chunks {r, 7-r} (zigzag balances causal attention work across ranks).
  Each core redundantly computes LN1 + K + V for its whole batch, then
  attention, projection, LN2 and the MLP for its own 512 tokens only.

v2 design (vs the v1 DRAM-bounce kernel):
  - Everything stays in SBUF: K^T [2-head row-blocks, T], V in per-head
    [s, d | ones] interleaved tiles (computed directly by making the LN1
    activations the stationary matmul operand), Q^T in head pairs.
  - All three LayerNorms run in the transposed [c, t] layout: column sums
    and sum-of-squares come from ones-vector matmuls on the PE, tiny [1, t]
    stat rows turn into (x + B)*R with B/R broadcast along partitions via
    K=1 matmuls; no PE transposes anywhere in the kernel.
  - Matmul operands are bf16 (PSUM accumulation stays fp32; tolerance is
    2e-2, measured ~2e-3); stats/residual stay fp32.
  - Causal masking is 0/1 host-fed bf16 tiles multiplied on the DVE (the v1
    gpsimd path spent ~0.5 ms in gpsimd semaphore overhead); exp runs as
    [128, 1024]-wide ACT ops; scores/exp/AV are software-pipelined one unit
    ahead and the per-head softmax tail (1/Z + broadcast) is deferred one
    head to keep the PE stream dense.
  - All reciprocals/rsqrts run on ScalarE as exp(-ln(x)) (DVE's iterative
    divide costs ~8 cyc/elem on one lane for [1,512] rows; Ln and Exp share
    one ACT table set so the whole kernel needs no table swaps; LN drops
    the +eps inside 1/(std+eps), a ~1e-6 relative effect).
  - Each core's token order is host-permuted so its own zigzag chunks sit
    at fixed positions 0 and 7; masks are built in permuted coordinates.
  - ~25 large DMAs total (v1: 265), weights host-packed to SBUF layout.
  - LN1 runs fully in bf16 (x input, stats operands, SBUF bf16 broadcast
    tiles) so its DVE apply ops hit the 2x packed mode; attention processes
    two heads interleaved through the score/exp/mask/AV pipeline so the PE
    always has the sibling head's matmuls during ACT/DVE latency.
  - wqk weight DMA prefetched under LN1; MLP weight pool triple-buffered
    so the l3 weight load overlaps l1 compute.
  Measured on TRN2 (NTFF profile): 565 us vs 928 us for v1, rel err 8e-4.
  Remaining known headroom: PE sits at half clock (HAM K=4/8) ~47% of the
  time; quad-wise K/V/LN1 sharing via shared-DRAM all-gather (~-80us), an
  interleaved token->core assignment (-17% exp / -50% mask volume), and
  fp8 DoubleRow for the MLP (~-50us) are the next levers.

One NEFF runs on all 8 cores: attention uses a fixed union schedule
(q-half 0: s-chunks 0..3, q-half 1: s-chunks 0..7) and host-fed masks gate
inactive chunks and diagonal triangles.
"""

import numpy as np

B, T, C = 2, 2048, 1024
H, D = 16, 64
FF = 4 * C
EPS = 1e-6
N_CORES = 8
NCHUNK = 8
CH = T // NCHUNK        # 256 tokens per causal chunk
RANKS = 4
OWN = T // RANKS        # 512 tokens owned per core
P = 128
NB = 512                # matmul moving-dim tile
KC = C // P             # 8 contraction chunks over C
TB = T // NB            # 4 column blocks over T
FB = FF // P            # 32 ff row blocks
VW = D + 1              # per-head V columns incl. ones column
HP = H // 2             # head pairs


def build_core_program(nc, tile, mybir, n_iters=1):
    from contextlib import ExitStack

    dt = mybir.dt
    f32 = dt.float32
    f32r = dt.float32r
    bf16 = dt.bfloat16
    AF = mybir.ActivationFunctionType
    ALU = mybir.AluOpType

    # ---- inputs, all host-packed to the exact SBUF layout ----
    xT_in = nc.dram_tensor("xT_in", [P, KC * T], bf16, kind="ExternalInput").ap()
    xTown_in = nc.dram_tensor("xTown_in", [P, KC * OWN], f32r,
                              kind="ExternalInput").ap()
    wqk_in = nc.dram_tensor("wqk_in", [P, 2 * KC * C], bf16,
                            kind="ExternalInput").ap()
    wv_in = nc.dram_tensor("wv_in", [P, KC * C], bf16,
                           kind="ExternalInput").ap()
    wproj_in = nc.dram_tensor("wproj_in", [P, KC * C], bf16,
                              kind="ExternalInput").ap()
    wl1_in = nc.dram_tensor("wl1_in", [P, FB * C], bf16,
                            kind="ExternalInput").ap()
    wl3_in = nc.dram_tensor("wl3_in", [P, KC * FF], bf16,
                            kind="ExternalInput").ap()
    bias_in = nc.dram_tensor("bias_in", [P, 64], f32, kind="ExternalInput").ap()
    bv_in = nc.dram_tensor("bv_in", [P, C], f32, kind="ExternalInput").ap()
    mask_in = nc.dram_tensor("mask_in", [P, NCHUNK * NB], bf16,
                             kind="ExternalInput").ap()
    outT = nc.dram_tensor("outT", [C, OWN], f32, kind="ExternalOutput").ap()

    with tile.TileContext(nc) as tc, ExitStack() as ctx:
        cpool = ctx.enter_context(tc.tile_pool(name="const", bufs=1))
        ones_col_f = cpool.tile([P, 1], f32, name="ones_col_f")
        nc.vector.memset(ones_col_f[:], 1.0)
        ones_row_f = cpool.tile([1, P], f32, name="ones_row_f")
        nc.vector.memset(ones_row_f[:], 1.0)
        negones_row_f = cpool.tile([1, P], f32, name="negones_row_f")
        nc.vector.memset(negones_row_f[:], -1.0)
        ones_col_b = cpool.tile([P, 1], bf16, name="ones_col_b")
        nc.vector.memset(ones_col_b[:], 1.0)
        ones_col = ones_col_f[:].bitcast(f32r)
        ones_row = ones_row_f[:].bitcast(f32r)
        negones_row = negones_row_f[:].bitcast(f32r)
        biases = cpool.tile([P, 64], f32, name="biases")
        nc.sync.dma_start(biases[:], bias_in)
        bq = biases[:, 0:8]
        bk = biases[:, 8:16]
        bproj = biases[:, 16:24]
        bl1 = biases[:, 24:56]
        bl3 = biases[:, 56:64]

        stat = ctx.enter_context(tc.tile_pool(name="stat", bufs=1))

        def ln_stats(src, cols, n, work, pstat, pbc, ones_in=None,
                     sqdt=None, bc_sbuf_bf=False):
            """Column stats of src[:, cols(kc, n)] over all KC chunks.
            Returns (bcB, bcR): xhat = (x + bcB) * bcR. With bc_sbuf_bf the
            broadcasts are copied to SBUF bf16 so the apply ops hit the DVE
            2x packed mode."""
            if ones_in is None:
                ones_in = ones_col
            if sqdt is None:
                sqdt = f32r
            s1 = pstat.tile([1, n], f32, name="s1")
            s2 = pstat.tile([1, n], f32, name="s2")
            for kc in range(KC):
                nc.tensor.matmul(s1[:], ones_in, src[:, cols(kc)],
                                 start=(kc == 0), stop=(kc == KC - 1))
            for kc in range(KC):
                sq = work.tile([P, n], sqdt, name="sq")
                nc.scalar.activation(sq[:], src[:, cols(kc)], AF.Square)
                nc.tensor.matmul(s2[:], ones_in, sq[:],
                                 start=(kc == 0), stop=(kc == KC - 1))
            # m2 = s1^2/C via Square's input scale (1/sqrt(C) = 2^-5 exact)
            m2 = stat.tile([1, n], f32, name="m2")
            nc.scalar.activation(m2[:], s1[:], AF.Square, scale=C ** -0.5)
            dv = stat.tile([1, n], f32, name="dv")
            nc.vector.tensor_sub(dv[:], s2[:], m2[:])
            # 1/(std+eps) ~= rsqrt(var) = exp(-0.5*ln(var)); eps=1e-6 noise
            lnv = stat.tile([1, n], f32, name="lnv")
            nc.scalar.activation(lnv[:], dv[:], AF.Ln, scale=1.0 / (C - 1))
            rrr = stat.tile([1, n], f32r, name="rrr")
            nc.scalar.activation(rrr[:], lnv[:], AF.Exp, scale=-0.5)
            mr = stat.tile([1, n], f32r, name="mr")
            nc.vector.scalar_tensor_tensor(mr[:], s1[:], 1.0 / C, rrr[:],
                                           ALU.mult, ALU.mult)
            bcB = pbc.tile([P, n], f32, name="bcB")
            nc.tensor.matmul(bcB[:], negones_row, mr[:], start=True,
                             stop=True)
            bcR = pbc.tile([P, n], f32, name="bcR")
            nc.tensor.matmul(bcR[:], ones_row, rrr[:], start=True, stop=True)
            if not bc_sbuf_bf:
                return bcB, bcR
            bcBs = work.tile([P, n], bf16, name="bcBs")
            nc.vector.tensor_copy(bcBs[:], bcB[:])
            bcRs = work.tile([P, n], bf16, name="bcRs")
            nc.vector.tensor_copy(bcRs[:], bcR[:])
            return bcBs, bcRs

        def ln_apply(dst, dcols, src, scols, bcB, bcR, n, work, tdt=f32):
            for kc in range(KC):
                t1 = work.tile([P, n], tdt, name="t1")
                nc.vector.tensor_add(t1[:], src[:, scols(kc)], bcB[:])
                nc.vector.tensor_mul(dst[:, dcols(kc)], t1[:], bcR[:])


        def body(ctx2):
            x1p = ctx2.enter_context(tc.tile_pool(name="x1p", bufs=1))
            x1T = x1p.tile([P, KC * OWN], f32r, name="x1T")

            with tc.tile_pool(name="atp", bufs=1) as atp:
                aT = atp.tile([P, KC * OWN], bf16, name="aT")
                with tc.tile_pool(name="kvp", bufs=1) as kvp:
                    kT = kvp.tile([P, HP * T], bf16, name="kT")
                    vt = kvp.tile([P, 2 * NCHUNK * H * VW], bf16, name="vt")
                    qT = kvp.tile([P, HP * OWN], bf16, name="qT")
                    nc.vector.memset(
                        vt[:].rearrange("p (s h w) -> p s h w",
                                        s=16, h=H)[:, :, :, D:D + 1],
                        1.0)

                    with tc.tile_pool(name="ln1p", bufs=1) as ln1p, \
                         tc.tile_pool(name="wqkp", bufs=1) as wqkp:
                        ln1T = ln1p.tile([P, KC * T], bf16, name="ln1T")
                        wqk = wqkp.tile([P, 2 * KC * C], bf16, name="wqk")
                        nc.sync.dma_start(wqk[:], wqk_in)

                        # ---- LN1 + QKV share one PSUM scope so the
                        # QKV matmuls can overlap the LN1 tail (a separate
                        # pool would barrier on LN1's full PSUM release) ----
                        with tc.tile_pool(name="pstat", bufs=1,
                                          space="PSUM") as pstat, \
                             tc.tile_pool(name="pbc", bufs=1,
                                          space="PSUM") as pbc, \
                             tc.tile_pool(name="ps", bufs=4,
                                          space="PSUM") as psp:
                          with tc.tile_pool(name="xtp", bufs=2) as xtp, \
                               tc.tile_pool(name="lnw", bufs=3) as lnw:
                            xT_src = xT_in.rearrange("p (kc t) -> p kc t",
                                                     kc=KC)
                            for nb in (0, TB - 1, 1, 2):
                                xw = xtp.tile([P, KC * NB], bf16, name="xw")
                                nc.sync.dma_start(
                                    xw[:].rearrange("p (kc t) -> p kc t",
                                                    kc=KC),
                                    xT_src[:, :, nb * NB:(nb + 1) * NB])
                                bcB, bcR = ln_stats(
                                    xw,
                                    lambda kc: slice(kc * NB, (kc + 1) * NB),
                                    NB, lnw, pstat, pbc,
                                    ones_in=ones_col_b[:], sqdt=bf16,
                                    bc_sbuf_bf=True)
                                ln_apply(
                                    ln1T,
                                    lambda kc: slice(kc * T + nb * NB,
                                                     kc * T + nb * NB + NB),
                                    xw,
                                    lambda kc: slice(kc * NB, (kc + 1) * NB),
                                    bcB, bcR, NB, lnw, tdt=bf16)

                          # ---- K^T, V, Q^T (all SBUF-resident) ----
                          with tc.tile_pool(name="wvp", bufs=1) as wvp:
                            wv = wvp.tile([P, KC * C], bf16, name="wv")
                            nc.sync.dma_start(wv[:], wv_in)
                            vb = wvp.tile([P, C], f32, name="vb")
                            nc.sync.dma_start(vb[:], bv_in)

                            # Q: 8 head pairs x (2 own chunks x 8 kc), N=256
                            for m in range(HP):
                                for cc in range(2):
                                    ps = psp.tile([P, NB], f32, name="ps")
                                    for kc in range(KC):
                                        nc.tensor.matmul(
                                            ps[:, 0:CH],
                                            wqk[:, m * C + kc * P:
                                                m * C + kc * P + P],
                                            ln1T[:, kc * T
                                                 + OWN_CHUNK_OFF[cc] * CH:
                                                 kc * T
                                                 + OWN_CHUNK_OFF[cc] * CH
                                                 + CH],
                                            start=(kc == 0),
                                            stop=(kc == KC - 1))
                                    nc.vector.tensor_scalar_add(
                                        qT[:, m * OWN + cc * CH:
                                           m * OWN + (cc + 1) * CH],
                                        ps[:, 0:CH], bq[:, m:m + 1])

                            # K: 8 pair-blocks x 4 nb x 8 kc, N=512
                            for m in range(HP):
                                for nb in range(TB):
                                    ps = psp.tile([P, NB], f32, name="ps")
                                    for kc in range(KC):
                                        nc.tensor.matmul(
                                            ps[:],
                                            wqk[:, (HP + m) * C + kc * P:
                                                (HP + m) * C + kc * P + P],
                                            ln1T[:, kc * T + nb * NB:
                                                 kc * T + (nb + 1) * NB],
                                            start=(kc == 0),
                                            stop=(kc == KC - 1))
                                    nc.vector.tensor_scalar_add(
                                        kT[:, m * T + nb * NB:
                                           m * T + (nb + 1) * NB],
                                        ps[:], bk[:, m:m + 1])

                            # V: 16 s-blocks x 2 v-halves x 8 kc,
                            # activations stationary -> [s, d] layout
                            for sblk in range(2 * NCHUNK):
                                for vh in range(2):
                                    ps = psp.tile([P, NB], f32, name="ps")
                                    for kc in range(KC):
                                        nc.tensor.matmul(
                                            ps[:],
                                            ln1T[:, kc * T + sblk * P:
                                                 kc * T + (sblk + 1) * P],
                                            wv[:, kc * C + vh * NB:
                                               kc * C + (vh + 1) * NB],
                                            start=(kc == 0),
                                            stop=(kc == KC - 1))
                                    dst = vt[:, sblk * H * VW + vh * 8 * VW:
                                             sblk * H * VW
                                             + (vh + 1) * 8 * VW]
                                    nc.vector.tensor_add(
                                        dst.rearrange("p (h w) -> p h w",
                                                      h=8)[:, :, 0:D],
                                        ps[:].rearrange("p (h d) -> p h d",
                                                        h=8),
                                        vb[:, vh * NB:(vh + 1) * NB]
                                        .rearrange("p (h d) -> p h d", h=8))

                    # ---- attention ----
                    with tc.tile_pool(name="mkp", bufs=1) as mkp, \
                         tc.tile_pool(name="exp_", bufs=4) as exp_, \
                         tc.tile_pool(name="att", bufs=3) as att, \
                         tc.tile_pool(name="psc", bufs=2,
                                      space="PSUM") as psc, \
                         tc.tile_pool(name="pav", bufs=3,
                                      space="PSUM") as pav, \
                         tc.tile_pool(name="pbc2", bufs=1,
                                      space="PSUM") as pbc2:
                        masks = mkp.tile([P, NCHUNK * NB], bf16, name="masks")
                        nc.sync.dma_start(masks[:], mask_in)

                        def emit_unit(h, u):
                            """Scores + exp + mask for one unit; returns ex.
                            Units 0-3: s-chunk u, both q-halves (N=512/sb).
                            Units 4-5: s-chunk pair (8+2(u-4), 9+2(u-4)),
                            q-half 1 only."""
                            hb, ho = h // 2, (h % 2) * D
                            qt = qT[ho:ho + D, hb * OWN:(hb + 1) * OWN]
                            ps = psc.tile([P, 2 * NB], f32, name="ps")
                            ex = exp_.tile([P, 2 * NB], bf16, name="ex")
                            if u < 4:
                                sc = u
                                for sb in range(2):
                                    nc.tensor.matmul(
                                        ps[:, sb * NB:(sb + 1) * NB],
                                        kT[ho:ho + D,
                                           hb * T + sc * CH + sb * P:
                                           hb * T + sc * CH + sb * P + P],
                                        qt, start=True, stop=True)
                                nc.scalar.activation(ex[:], ps[:], AF.Exp)
                                exv = ex[:].rearrange(
                                    "p (sb qh q) -> p sb qh q",
                                    sb=2, qh=2)[:, :, 0, :]
                                mkv = masks[:, sc * NB:(sc + 1) * NB] \
                                    .rearrange("p (sb q) -> p sb q", sb=2)
                                nc.vector.tensor_mul(exv, exv, mkv)
                            else:
                                sc0 = 4 + 2 * (u - 4)
                                for j in range(2):     # chunk sc0+j
                                    for sb in range(2):
                                        nc.tensor.matmul(
                                            ps[:, j * NB + sb * CH:
                                               j * NB + (sb + 1) * CH],
                                            kT[ho:ho + D,
                                               hb * T + (sc0 + j) * CH
                                               + sb * P:
                                               hb * T + (sc0 + j) * CH
                                               + sb * P + P],
                                            qt[:, CH:2 * CH],
                                            start=(sb == 0), stop=(sb == 1))
                                nc.scalar.activation(ex[:], ps[:], AF.Exp)
                                nc.vector.tensor_mul(
                                    ex[:], ex[:],
                                    masks[:, sc0 * NB:(sc0 + 2) * NB])
                            return ex

                        def emit_av(h, u, ex, av):
                            for i in range(2 if u < 4 else 4):
                                if u < 4:
                                    sc, sb = u, i
                                    rhsA = ex[:, sb * NB:sb * NB + CH]
                                    rhsB = ex[:, sb * NB + CH:
                                              sb * NB + 2 * CH]
                                else:
                                    sc = 4 + 2 * (u - 4) + i // 2
                                    sb = i % 2
                                    rhsA = None
                                    rhsB = ex[:, (i // 2) * NB + sb * CH:
                                              (i // 2) * NB + (sb + 1) * CH]
                                vslice = vt[:, (sc * 2 + sb) * H * VW
                                            + h * VW:
                                            (sc * 2 + sb) * H * VW
                                            + (h + 1) * VW]
                                if rhsA is not None:
                                    nc.tensor.matmul(
                                        av[:, 0:CH], vslice, rhsA,
                                        start=(u == 0 and i == 0),
                                        stop=(u == 3 and i == 1),
                                        skip_group_check=True)
                                nc.tensor.matmul(
                                    av[:, CH:2 * CH], vslice, rhsB,
                                    start=False,
                                    stop=(u == 5 and i == 3),
                                    skip_group_check=True)

                        def emit_tail(h, av):
                            hb, ho = h // 2, (h % 2) * D
                            lnz = att.tile([1, 2 * CH], f32, name="lnz")
                            nc.scalar.activation(lnz[:], av[D:D + 1, :],
                                                 AF.Ln)
                            rzr = att.tile([1, 2 * CH], f32r, name="rzr")
                            nc.scalar.activation(rzr[:], lnz[:], AF.Exp,
                                                 scale=-1.0)
                            bc = pbc2.tile([D, 2 * CH], f32, name="bc")
                            nc.tensor.matmul(bc[:], ones_row[0:1, 0:D],
                                             rzr[:], start=True, stop=True)
                            bcs = att.tile([D, 2 * CH], f32, name="bcs")
                            nc.vector.tensor_copy(bcs[:], bc[:])
                            nc.vector.tensor_mul(
                                aT[ho:ho + D, hb * OWN:(hb + 1) * OWN],
                                av[0:D, :], bcs[:])

                        # Two heads run interleaved through the unit
                        # pipeline (PE always has the sibling head's matmuls
                        # while ACT/DVE chew exp+mask); AV trails by two
                        # pipeline slots; head tails: first at pair end,
                        # second deferred into the next pair.
                        prev = None        # (h, av) awaiting tail
                        for hp in range(0, H, 2):
                            avs = {hp: pav.tile([VW, 2 * CH], f32,
                                                name="av"),
                                   hp + 1: pav.tile([VW, 2 * CH], f32,
                                                    name="av")}
                            pend = []      # (h, u, ex) awaiting AV
                            for u in range(6):
                                for h in (hp, hp + 1):
                                    ex = emit_unit(h, u)
                                    if len(pend) >= 2:
                                        ph, pu, pex = pend.pop(0)
                                        emit_av(ph, pu, pex, avs[ph])
                                    pend.append((h, u, ex))
                                if u == 2 and prev is not None:
                                    emit_tail(*prev)
                                    prev = None
                            for ph, pu, pex in pend:
                                emit_av(ph, pu, pex, avs[ph])
                            emit_tail(hp, avs[hp])
                            prev = (hp + 1, avs[hp + 1])
                        emit_tail(*prev)

                # ---- proj + residual -> x1T (fp32) ----
                with tc.tile_pool(name="wpp", bufs=1) as wpp, \
                     tc.tile_pool(name="evp", bufs=4) as evp, \
                     tc.tile_pool(name="psp2", bufs=4, space="PSUM") as psp2:
                    wproj = wpp.tile([P, KC * C], bf16, name="wproj")
                    nc.sync.dma_start(wproj[:], wproj_in)
                    xTo = wpp.tile([P, KC * OWN], f32r, name="xTo")
                    nc.sync.dma_start(xTo[:], xTown_in)
                    for m in range(KC):
                        ps = psp2.tile([P, NB], f32, name="ps")
                        for kc in range(KC):
                            nc.tensor.matmul(
                                ps[:],
                                wproj[:, m * C + kc * P:m * C + kc * P + P],
                                aT[:, kc * OWN:(kc + 1) * OWN],
                                start=(kc == 0), stop=(kc == KC - 1))
                        nc.vector.scalar_tensor_tensor(
                            x1T[:, m * OWN:(m + 1) * OWN], ps[:],
                            bproj[:, m:m + 1],
                            xTo[:, m * OWN:(m + 1) * OWN],
                            ALU.add, ALU.add)

            # ---- LN2 + MLP ----
            with tc.tile_pool(name="mlpw", bufs=3) as mlpw, \
                 tc.tile_pool(name="ln2p", bufs=1) as ln2p, \
                 tc.tile_pool(name="hp", bufs=1) as hp, \
                 tc.tile_pool(name="lnw2", bufs=3) as lnw2, \
                 tc.tile_pool(name="evp2", bufs=4) as evp2, \
                 tc.tile_pool(name="pstat2", bufs=1, space="PSUM") as pstat2, \
                 tc.tile_pool(name="pbc3", bufs=1, space="PSUM") as pbc3, \
                 tc.tile_pool(name="psp3", bufs=3, space="PSUM") as psp3:
                ln2T = ln2p.tile([P, KC * OWN], bf16, name="ln2T")
                bcB, bcR = ln_stats(
                    x1T, lambda kc: slice(kc * OWN, (kc + 1) * OWN),
                    OWN, lnw2, pstat2, pbc3)
                ln_apply(ln2T, lambda kc: slice(kc * OWN, (kc + 1) * OWN),
                         x1T, lambda kc: slice(kc * OWN, (kc + 1) * OWN),
                         bcB, bcR, OWN, lnw2)

                hT = hp.tile([P, FB * OWN], bf16, name="hT")
                for half in range(2):
                    wl1 = mlpw.tile([P, FB * C // 2], bf16, name="wl1",
                                    tag="w")
                    nc.sync.dma_start(
                        wl1[:], wl1_in[:, half * FB * C // 2:
                                       (half + 1) * FB * C // 2])
                    for mm_ in range(FB // 2):
                        m = half * (FB // 2) + mm_
                        ps = psp3.tile([P, NB], f32, name="ps")
                        for kc in range(KC):
                            nc.tensor.matmul(
                                ps[:],
                                wl1[:, mm_ * C + kc * P:mm_ * C + kc * P + P],
                                ln2T[:, kc * OWN:(kc + 1) * OWN],
                                start=(kc == 0), stop=(kc == KC - 1))
                        nc.scalar.activation(hT[:, m * OWN:(m + 1) * OWN],
                                             ps[:], AF.Relu,
                                             bias=bl1[:, m:m + 1])

                for half in range(2):
                    wl3 = mlpw.tile([P, KC * FF // 2], bf16, name="wl3",
                                    tag="w")
                    nc.sync.dma_start(
                        wl3[:], wl3_in[:, half * KC * FF // 2:
                                       (half + 1) * KC * FF // 2])
                    for mm_ in range(KC // 2):
                        m = half * (KC // 2) + mm_
                        ps = psp3.tile([P, NB], f32, name="ps")
                        for fc in range(FB):
                            nc.tensor.matmul(
                                ps[:],
                                wl3[:, mm_ * FF + fc * P:
                                    mm_ * FF + fc * P + P],
                                hT[:, fc * OWN:(fc + 1) * OWN],
                                start=(fc == 0), stop=(fc == FB - 1))
                        o = evp2.tile([P, NB], f32, name="o")
                        nc.vector.scalar_tensor_tensor(
                            o[:], ps[:], bl3[:, m:m + 1],
                            x1T[:, m * OWN:(m + 1) * OWN],
                            ALU.add, ALU.add)
                        nc.sync.dma_start(outT[m * P:(m + 1) * P, :], o[:])

        if n_iters == 1:
            with ExitStack() as ctx2:
                body(ctx2)
        else:
            with tc.For_i(0, n_iters, 1):
                with ExitStack() as ctx2:
                    body(ctx2)

    return nc


# The host permutes each core's token order so its own zigzag chunks
# always sit at chunk positions 0 (lo) and 7 (hi); Q then reads ln1T at
# fixed columns, and masks are host-built in the permuted coordinates.
OWN_CHUNK_OFF = (0, 7)


def _pack_lhsT(w):
    """[M, K] weight (out = w @ x) -> [128, (M/128)*(K/128)*128] with
    [p, m*K + kc*128 + j] = w[m*128 + j, kc*128 + p]."""
    M, K = w.shape
    nm, nk = M // P, K // P
    return np.ascontiguousarray(
        w.reshape(nm, P, nk, P).transpose(3, 0, 2, 1).reshape(P, nm * K))


def _host_prep(x, qkv_w, proj_w, proj_b, l1_w, l1_b, l3_w, l3_b,
               ln1_g, ln1_b, ln2_g, ln2_b):
    import ml_dtypes
    f = np.float32
    bf = ml_dtypes.bfloat16
    x = np.asarray(x, f)
    qkv_w = np.asarray(qkv_w, f)
    scale = np.float32(D ** -0.5)
    w_eff = qkv_w * np.asarray(ln1_g, f)[None, :]
    b_eff = (qkv_w @ np.asarray(ln1_b, f)).astype(f)
    w_eff[:C] *= scale
    b_eff[:C] *= scale
    l1_eff = np.asarray(l1_w, f) * np.asarray(ln2_g, f)[None, :]
    bl1_eff = (np.asarray(l1_b, f)
               + np.asarray(l1_w, f) @ np.asarray(ln2_b, f)).astype(f)

    # head-pair biases [128, m]: [p, m] = b[m*128 + p]
    def colpack(b):
        return np.asarray(b, f).reshape(-1, P).T  # [128, nm]

    bias_all = np.zeros((P, 64), f)
    bias_all[:, 0:8] = colpack(b_eff[0:C])
    bias_all[:, 8:16] = colpack(b_eff[C:2 * C])
    bias_all[:, 16:24] = colpack(np.asarray(proj_b, f))
    bias_all[:, 24:56] = colpack(bl1_eff)
    bias_all[:, 56:64] = colpack(np.asarray(l3_b, f))

    shared = {
        "wqk_in": _pack_lhsT(w_eff[0:2 * C]).astype(bf),
        # V rhs pack: [p, kc*C + vcol] = w_eff[2C + vcol, kc*128 + p]
        "wv_in": np.ascontiguousarray(
            w_eff[2 * C:3 * C].reshape(C, KC, P).transpose(2, 1, 0)
            .reshape(P, KC * C)).astype(bf),
        "wproj_in": _pack_lhsT(np.asarray(proj_w, f)).astype(bf),
        "wl1_in": _pack_lhsT(l1_eff).astype(bf),
        "wl3_in": _pack_lhsT(np.asarray(l3_w, f)).astype(bf),
        "bias_in": bias_all,
        "bv_in": np.ascontiguousarray(
            np.broadcast_to(b_eff[2 * C:3 * C][None, :], (P, C))),
    }

    in_maps = []
    for cid in range(N_CORES):
        b, r = divmod(cid, RANKS)
        lo, hi = r, NCHUNK - 1 - r
        # permute token chunks so own chunks sit at positions 0 and 7:
        # perm[newchunk] = oldchunk; own lo -> 0, own hi -> 7, others keep
        # relative order in between.
        rest = [c for c in range(NCHUNK) if c not in (lo, hi)]
        perm = [lo] + rest + [hi]
        old_order = np.asarray(perm)
        tok_perm = (old_order[:, None] * CH
                    + np.arange(CH)[None, :]).reshape(-1)  # new idx -> old tok
        xb = x[b][tok_perm]                                # [T, C] permuted
        # xT layout [p, kc*T + t] = xb[t, kc*128 + p]
        xT_in = np.ascontiguousarray(
            xb.T.reshape(KC, P, T).transpose(1, 0, 2).reshape(P, KC * T)
        ).astype(bf)
        x_own = xb[np.r_[0:CH, (NCHUNK - 1) * CH:T]]       # chunks 0,7 = own
        xTown_in = np.ascontiguousarray(
            x_own.T.reshape(KC, P, OWN).transpose(1, 0, 2).reshape(P, KC * OWN))

        # masks in PERMUTED coordinates: new chunk nc_ holds old chunk
        # perm[nc_]; query chunks: qh0 = new 0 (old lo), qh1 = new 7 (old hi).
        # causal: old_s_token <= old_q_token.
        m2 = np.zeros((NCHUNK, P, 2, CH), f)
        for nsc in range(NCHUNK):
            osc = perm[nsc]
            for sb in range(2):
                s_old = osc * CH + sb * P + np.arange(P)  # [128]
                if nsc < 4:
                    q_old = lo * CH + np.arange(CH)
                else:
                    q_old = hi * CH + np.arange(CH)
                m2[nsc, :, sb, :] = (s_old[:, None] <= q_old[None, :])
        mask_in = np.ascontiguousarray(
            m2.transpose(1, 0, 2, 3).reshape(P, NCHUNK * NB)).astype(bf)

        in_maps.append({
            "xT_in": xT_in,
            "xTown_in": xTown_in,
            "mask_in": mask_in,
            **shared,
        })
    return in_maps


def _assemble(results):
    out = np.empty((B, T, C), np.float32)
    for cid in range(N_CORES):
        b, r = divmod(cid, RANKS)
        lo, hi = r, NCHUNK - 1 - r
        oT = results[cid]["outT"]
        out[b, lo * CH:(lo + 1) * CH] = oT[:, 0:CH].T
        out[b, hi * CH:(hi + 1) * CH] = oT[:, CH:2 * CH].T
    return out


_CACHE = {}


def get_nc(n_iters=1):
    if n_iters not in _CACHE:
        import concourse.bacc as bacc
        import concourse.tile as tile
        from concourse import mybir
        nc = bacc.Bacc("TRN2", target_bir_lowering=False, debug=False,
                       num_devices=N_CORES)
        build_core_program(nc, tile, mybir, n_iters=n_iters)
        nc.compile()
        _CACHE[n_iters] = nc
    return _CACHE[n_iters]


def run(inputs, n_iters=1):
    from concourse.bass_utils import run_bass_kernel_spmd
    in_maps = _host_prep(**inputs)
    nc = get_nc(n_iters)
    res = run_bass_kernel_spmd(nc, in_maps, list(range(N_CORES)))
    return _assemble(res.results)


def kernel(**inputs):
    return run(inputs, n_iters=1)



# revision 16
# speedup vs baseline: 2.0177x; 2.0177x over previous
"""Trainium2 Bass kernel for a dense transformer block (B=2, T=2048, C=1024,
H=16, D=64, FF=4096), SPMD on 8 NeuronCores.

Sharding: pure data-parallel over tokens, zero collectives.
  core cid -> batch b = cid // 4, rank r = cid % 4.
  Each batch's 2048 tokens split into 8 causal chunks of 256; rank r owns
  chunks {r, 7-r} (zigzag balances causal attention work across ranks).
  Each core redundantly computes LN1 + K + V for its whole batch, then
  attention, projection, LN2 and the MLP for its own 512 tokens only.

v2 design (vs the v1 DRAM-bounce kernel):
  - Everything stays in SBUF: K^T [2-head row-blocks, T], V in per-head
    [s, d | ones] interleaved tiles (computed directly by making the LN1
    activations the stationary matmul operand), Q^T in head pairs.
  - All three LayerNorms run in the transposed [c, t] layout: column sums
    and sum-of-squares come from ones-vector matmuls on the PE, tiny [1, t]
    stat rows turn into (x + B)*R with B/R broadcast along partitions via
    K=1 matmuls; no PE transposes anywhere in the kernel.
  - Matmul operands are bf16 (PSUM accumulation stays fp32; tolerance is
    2e-2, measured ~2e-3); stats/residual stay fp32.
  - Causal masking is 0/1 host-fed bf16 tiles multiplied on the DVE (the v1
    gpsimd path spent ~0.5 ms in gpsimd semaphore overhead); exp runs as
    [128, 1024]-wide ACT ops; scores/exp/AV are software-pipelined one unit
    ahead and the per-head softmax tail (1/Z + broadcast) is deferred one
    head to keep the PE stream dense.
  - All reciprocals/rsqrts run on ScalarE as exp(-ln(x)) (DVE's iterative
    divide costs ~8 cyc/elem on one lane for [1,512] rows; Ln and Exp share
    one ACT table set so the whole kernel needs no table swaps; LN drops
    the +eps inside 1/(std+eps), a ~1e-6 relative effect).
  - Each core's token order is host-permuted so its own zigzag chunks sit
    at fixed positions 0 and 7; masks are built in permuted coordinates.
  - ~25 large DMAs total (v1: 265), weights host-packed to SBUF layout.
  - LN1 runs fully in bf16 (x input, stats operands, SBUF bf16 broadcast
    tiles) so its DVE apply ops hit the 2x packed mode; attention processes
    two heads interleaved through the score/exp/mask/AV pipeline so the PE
    always has the sibling head's matmuls during ACT/DVE latency.
  - wqk weight DMA prefetched under LN1; MLP weight pool triple-buffered
    so the l3 weight load overlaps l1 compute.

v5 changes on top of v2:
  - AV matmuls for units 0..3 merged to one [65, 512] matmul per
    (unit, s-block) -- qh0/qh1 columns are adjacent in both ex and av.
  - LN-stats Square runs on the DVE (tensor_mul x*x, bf16 2x packed)
    instead of ACT, and m2/mr read an SBUF bounce of s1 (DVE ops may read
    only one PSUM input); keeps ScalarE's table on the Exp/Ln set.
  - Softmax-tail 1/Z negation moved to the DVE so the tail Exp runs at
    scale=1 like the unit exps (a scaled Exp reprograms the ACT table).
  Measured on TRN2 (NTFF profile, core 0): ~562 us, rel err 8e-4.

Paths measured and REJECTED (see session notes):
  - Quad-wise K/V/LN1 sharing via intra-quad DRAM AllGather: works
    (rel err unchanged) but each ~1MB collective costs ~60us of mesh
    protocol latency and two of them serialize on the gpsimd queue;
    only ~15us of independent work (Q) exists to hide it -> 630us.
  - Batching softmax tails per head pair (one Ln/Exp over [1,1024]):
    puts the 4us tail chain on the inter-pair critical path -> 661us.
  - fp8e4m3 DoubleRow MLP: per-dot relative error ~4% puts ~1.5e-2 on
    the output against a 2e-2 gate -- too close.
  Remaining known headroom: attention is ScalarE-bound (96 exp ops,
  ~1.05us each) and the PE sits at HAM K=4/8 for ~174us of it; ~16
  ACT-table reloads (Ln<->Exp do NOT share a table set) remain in the
  tails; cross-iteration software pipelining would hide the prefix.

One NEFF runs on all 8 cores: attention uses a fixed union schedule
(q-half 0: s-chunks 0..3, q-half 1: s-chunks 0..7) and host-fed masks gate
inactive chunks and diagonal triangles.
"""

import numpy as np

B, T, C = 2, 2048, 1024
H, D = 16, 64
FF = 4 * C
EPS = 1e-6
N_CORES = 8
NCHUNK = 8
CH = T // NCHUNK        # 256 tokens per causal chunk
RANKS = 4
OWN = T // RANKS        # 512 tokens owned per core
P = 128
NB = 512                # matmul moving-dim tile
KC = C // P             # 8 contraction chunks over C
TB = T // NB            # 4 column blocks over T
FB = FF // P            # 32 ff row blocks
VW = D + 1              # per-head V columns incl. ones column
HP = H // 2             # head pairs


def build_core_program(nc, tile, mybir, n_iters=1):
    from contextlib import ExitStack

    dt = mybir.dt
    f32 = dt.float32
    f32r = dt.float32r
    bf16 = dt.bfloat16
    AF = mybir.ActivationFunctionType
    ALU = mybir.AluOpType

    # ---- inputs, all host-packed to the exact SBUF layout ----
    xT_in = nc.dram_tensor("xT_in", [P, KC * T], bf16, kind="ExternalInput").ap()
    xTown_in = nc.dram_tensor("xTown_in", [P, KC * OWN], f32r,
                              kind="ExternalInput").ap()
    wqk_in = nc.dram_tensor("wqk_in", [P, 2 * KC * C], bf16,
                            kind="ExternalInput").ap()
    wv_in = nc.dram_tensor("wv_in", [P, KC * C], bf16,
                           kind="ExternalInput").ap()
    wproj_in = nc.dram_tensor("wproj_in", [P, KC * C], bf16,
                              kind="ExternalInput").ap()
    wl1_in = nc.dram_tensor("wl1_in", [P, FB * C], bf16,
                            kind="ExternalInput").ap()
    wl3_in = nc.dram_tensor("wl3_in", [P, KC * FF], bf16,
                            kind="ExternalInput").ap()
    bias_in = nc.dram_tensor("bias_in", [P, 64], f32, kind="ExternalInput").ap()
    bv_in = nc.dram_tensor("bv_in", [P, C], f32, kind="ExternalInput").ap()
    mask_in = nc.dram_tensor("mask_in", [P, NCHUNK * NB], bf16,
                             kind="ExternalInput").ap()
    outT = nc.dram_tensor("outT", [C, OWN], f32, kind="ExternalOutput").ap()

    with tile.TileContext(nc) as tc, ExitStack() as ctx:
        cpool = ctx.enter_context(tc.tile_pool(name="const", bufs=1))
        ones_col_f = cpool.tile([P, 1], f32, name="ones_col_f")
        nc.vector.memset(ones_col_f[:], 1.0)
        ones_row_f = cpool.tile([1, P], f32, name="ones_row_f")
        nc.vector.memset(ones_row_f[:], 1.0)
        negones_row_f = cpool.tile([1, P], f32, name="negones_row_f")
        nc.vector.memset(negones_row_f[:], -1.0)
        ones_col_b = cpool.tile([P, 1], bf16, name="ones_col_b")
        nc.vector.memset(ones_col_b[:], 1.0)
        ones_col = ones_col_f[:].bitcast(f32r)
        ones_row = ones_row_f[:].bitcast(f32r)
        negones_row = negones_row_f[:].bitcast(f32r)
        biases = cpool.tile([P, 64], f32, name="biases")
        nc.sync.dma_start(biases[:], bias_in)
        bq = biases[:, 0:8]
        bk = biases[:, 8:16]
        bproj = biases[:, 16:24]
        bl1 = biases[:, 24:56]
        bl3 = biases[:, 56:64]

        stat = ctx.enter_context(tc.tile_pool(name="stat", bufs=1))

        def ln_stats(src, cols, n, work, pstat, pbc, ones_in=None,
                     sqdt=None, bc_sbuf_bf=False):
            """Column stats of src[:, cols(kc, n)] over all KC chunks.
            Returns (bcB, bcR): xhat = (x + bcB) * bcR. With bc_sbuf_bf the
            broadcasts are copied to SBUF bf16 so the apply ops hit the DVE
            2x packed mode."""
            if ones_in is None:
                ones_in = ones_col
            if sqdt is None:
                sqdt = f32r
            s1 = pstat.tile([1, n], f32, name="s1")
            s2 = pstat.tile([1, n], f32, name="s2")
            for kc in range(KC):
                nc.tensor.matmul(s1[:], ones_in, src[:, cols(kc)],
                                 start=(kc == 0), stop=(kc == KC - 1))
            for kc in range(KC):
                sq = work.tile([P, n], sqdt, name="sq")
                nc.vector.tensor_mul(sq[:], src[:, cols(kc)], src[:, cols(kc)])
                nc.tensor.matmul(s2[:], ones_in, sq[:],
                                 start=(kc == 0), stop=(kc == KC - 1))
            # m2 = s1^2/C on the DVE (keeps ScalarE on the Exp/Ln table set);
            # bounce s1 to SBUF first -- DVE ops may read only one PSUM input.
            s1s = stat.tile([1, n], f32, name="s1s")
            nc.vector.tensor_copy(s1s[:], s1[:])
            m2 = stat.tile([1, n], f32, name="m2")
            nc.vector.scalar_tensor_tensor(m2[:], s1s[:], 1.0 / C, s1s[:],
                                           ALU.mult, ALU.mult)
            dv = stat.tile([1, n], f32, name="dv")
            nc.vector.tensor_sub(dv[:], s2[:], m2[:])
            # 1/(std+eps) ~= rsqrt(var) = exp(-0.5*ln(var)); eps=1e-6 noise
            lnv = stat.tile([1, n], f32, name="lnv")
            nc.scalar.activation(lnv[:], dv[:], AF.Ln, scale=1.0 / (C - 1))
            rrr = stat.tile([1, n], f32r, name="rrr")
            nc.scalar.activation(rrr[:], lnv[:], AF.Exp, scale=-0.5)
            mr = stat.tile([1, n], f32r, name="mr")
            nc.vector.scalar_tensor_tensor(mr[:], s1s[:], 1.0 / C, rrr[:],
                                           ALU.mult, ALU.mult)
            bcB = pbc.tile([P, n], f32, name="bcB")
            nc.tensor.matmul(bcB[:], negones_row, mr[:], start=True,
                             stop=True)
            bcR = pbc.tile([P, n], f32, name="bcR")
            nc.tensor.matmul(bcR[:], ones_row, rrr[:], start=True, stop=True)
            if not bc_sbuf_bf:
                return bcB, bcR
            bcBs = work.tile([P, n], bf16, name="bcBs")
            nc.vector.tensor_copy(bcBs[:], bcB[:])
            bcRs = work.tile([P, n], bf16, name="bcRs")
            nc.vector.tensor_copy(bcRs[:], bcR[:])
            return bcBs, bcRs

        def ln_apply(dst, dcols, src, scols, bcB, bcR, n, work, tdt=f32):
            for kc in range(KC):
                t1 = work.tile([P, n], tdt, name="t1")
                nc.vector.tensor_add(t1[:], src[:, scols(kc)], bcB[:])
                nc.vector.tensor_mul(dst[:, dcols(kc)], t1[:], bcR[:])


        def body(ctx2):
            x1p = ctx2.enter_context(tc.tile_pool(name="x1p", bufs=1))
            x1T = x1p.tile([P, KC * OWN], f32r, name="x1T")

            with tc.tile_pool(name="atp", bufs=1) as atp:
                aT = atp.tile([P, KC * OWN], bf16, name="aT")
                with tc.tile_pool(name="kvp", bufs=1) as kvp:
                    kT = kvp.tile([P, HP * T], bf16, name="kT")
                    vt = kvp.tile([P, 2 * NCHUNK * H * VW], bf16, name="vt")
                    qT = kvp.tile([P, HP * OWN], bf16, name="qT")
                    nc.vector.memset(
                        vt[:].rearrange("p (s h w) -> p s h w",
                                        s=16, h=H)[:, :, :, D:D + 1],
                        1.0)

                    with tc.tile_pool(name="ln1p", bufs=1) as ln1p, \
                         tc.tile_pool(name="wqkp", bufs=1) as wqkp:
                        ln1T = ln1p.tile([P, KC * T], bf16, name="ln1T")
                        wqk = wqkp.tile([P, 2 * KC * C], bf16, name="wqk")
                        nc.sync.dma_start(wqk[:], wqk_in)

                        # ---- LN1 + QKV share one PSUM scope so the
                        # QKV matmuls can overlap the LN1 tail (a separate
                        # pool would barrier on LN1's full PSUM release) ----
                        with tc.tile_pool(name="pstat", bufs=1,
                                          space="PSUM") as pstat, \
                             tc.tile_pool(name="pbc", bufs=1,
                                          space="PSUM") as pbc, \
                             tc.tile_pool(name="ps", bufs=4,
                                          space="PSUM") as psp:
                          with tc.tile_pool(name="xtp", bufs=2) as xtp, \
                               tc.tile_pool(name="lnw", bufs=3) as lnw:
                            xT_src = xT_in.rearrange("p (kc t) -> p kc t",
                                                     kc=KC)
                            for nb in (0, TB - 1, 1, 2):
                                xw = xtp.tile([P, KC * NB], bf16, name="xw")
                                nc.sync.dma_start(
                                    xw[:].rearrange("p (kc t) -> p kc t",
                                                    kc=KC),
                                    xT_src[:, :, nb * NB:(nb + 1) * NB])
                                bcB, bcR = ln_stats(
                                    xw,
                                    lambda kc: slice(kc * NB, (kc + 1) * NB),
                                    NB, lnw, pstat, pbc,
                                    ones_in=ones_col_b[:], sqdt=bf16,
                                    bc_sbuf_bf=True)
                                ln_apply(
                                    ln1T,
                                    lambda kc: slice(kc * T + nb * NB,
                                                     kc * T + nb * NB + NB),
                                    xw,
                                    lambda kc: slice(kc * NB, (kc + 1) * NB),
                                    bcB, bcR, NB, lnw, tdt=bf16)

                          # ---- K^T, V, Q^T (all SBUF-resident) ----
                          with tc.tile_pool(name="wvp", bufs=1) as wvp:
                            wv = wvp.tile([P, KC * C], bf16, name="wv")
                            nc.sync.dma_start(wv[:], wv_in)
                            vb = wvp.tile([P, C], f32, name="vb")
                            nc.sync.dma_start(vb[:], bv_in)

                            # Q: 8 head pairs x (2 own chunks x 8 kc), N=256
                            for m in range(HP):
                                for cc in range(2):
                                    ps = psp.tile([P, NB], f32, name="ps")
                                    for kc in range(KC):
                                        nc.tensor.matmul(
                                            ps[:, 0:CH],
                                            wqk[:, m * C + kc * P:
                                                m * C + kc * P + P],
                                            ln1T[:, kc * T
                                                 + OWN_CHUNK_OFF[cc] * CH:
                                                 kc * T
                                                 + OWN_CHUNK_OFF[cc] * CH
                                                 + CH],
                                            start=(kc == 0),
                                            stop=(kc == KC - 1))
                                    nc.vector.tensor_scalar_add(
                                        qT[:, m * OWN + cc * CH:
                                           m * OWN + (cc + 1) * CH],
                                        ps[:, 0:CH], bq[:, m:m + 1])

                            # K: 8 pair-blocks x 4 nb x 8 kc, N=512
                            for m in range(HP):
                                for nb in range(TB):
                                    ps = psp.tile([P, NB], f32, name="ps")
                                    for kc in range(KC):
                                        nc.tensor.matmul(
                                            ps[:],
                                            wqk[:, (HP + m) * C + kc * P:
                                                (HP + m) * C + kc * P + P],
                                            ln1T[:, kc * T + nb * NB:
                                                 kc * T + (nb + 1) * NB],
                                            start=(kc == 0),
                                            stop=(kc == KC - 1))
                                    nc.vector.tensor_scalar_add(
                                        kT[:, m * T + nb * NB:
                                           m * T + (nb + 1) * NB],
                                        ps[:], bk[:, m:m + 1])

                            # V: 16 s-blocks x 2 v-halves x 8 kc,
                            # activations stationary -> [s, d] layout
                            for sblk in range(2 * NCHUNK):
                                for vh in range(2):
                                    ps = psp.tile([P, NB], f32, name="ps")
                                    for kc in range(KC):
                                        nc.tensor.matmul(
                                            ps[:],
                                            ln1T[:, kc * T + sblk * P:
                                                 kc * T + (sblk + 1) * P],
                                            wv[:, kc * C + vh * NB:
                                               kc * C + (vh + 1) * NB],
                                            start=(kc == 0),
                                            stop=(kc == KC - 1))
                                    dst = vt[:, sblk * H * VW + vh * 8 * VW:
                                             sblk * H * VW
                                             + (vh + 1) * 8 * VW]
                                    nc.vector.tensor_add(
                                        dst.rearrange("p (h w) -> p h w",
                                                      h=8)[:, :, 0:D],
                                        ps[:].rearrange("p (h d) -> p h d",
                                                        h=8),
                                        vb[:, vh * NB:(vh + 1) * NB]
                                        .rearrange("p (h d) -> p h d", h=8))

                    # ---- attention ----
                    with tc.tile_pool(name="mkp", bufs=1) as mkp, \
                         tc.tile_pool(name="exp_", bufs=4) as exp_, \
                         tc.tile_pool(name="att", bufs=3) as att, \
                         tc.tile_pool(name="psc", bufs=2,
                                      space="PSUM") as psc, \
                         tc.tile_pool(name="pav", bufs=3,
                                      space="PSUM") as pav, \
                         tc.tile_pool(name="pbc2", bufs=1,
                                      space="PSUM") as pbc2:
                        masks = mkp.tile([P, NCHUNK * NB], bf16, name="masks")
                        nc.sync.dma_start(masks[:], mask_in)

                        def emit_unit(h, u):
                            """Scores + exp + mask for one unit; returns ex.
                            Units 0-3: s-chunk u, both q-halves (N=512/sb).
                            Units 4-5: s-chunk pair (8+2(u-4), 9+2(u-4)),
                            q-half 1 only."""
                            hb, ho = h // 2, (h % 2) * D
                            qt = qT[ho:ho + D, hb * OWN:(hb + 1) * OWN]
                            ps = psc.tile([P, 2 * NB], f32, name="ps")
                            ex = exp_.tile([P, 2 * NB], bf16, name="ex")
                            if u < 4:
                                sc = u
                                for sb in range(2):
                                    nc.tensor.matmul(
                                        ps[:, sb * NB:(sb + 1) * NB],
                                        kT[ho:ho + D,
                                           hb * T + sc * CH + sb * P:
                                           hb * T + sc * CH + sb * P + P],
                                        qt, start=True, stop=True)
                                nc.scalar.activation(ex[:], ps[:], AF.Exp)
                                exv = ex[:].rearrange(
                                    "p (sb qh q) -> p sb qh q",
                                    sb=2, qh=2)[:, :, 0, :]
                                mkv = masks[:, sc * NB:(sc + 1) * NB] \
                                    .rearrange("p (sb q) -> p sb q", sb=2)
                                nc.vector.tensor_mul(exv, exv, mkv)
                            else:
                                sc0 = 4 + 2 * (u - 4)
                                for j in range(2):     # chunk sc0+j
                                    for sb in range(2):
                                        nc.tensor.matmul(
                                            ps[:, j * NB + sb * CH:
                                               j * NB + (sb + 1) * CH],
                                            kT[ho:ho + D,
                                               hb * T + (sc0 + j) * CH
                                               + sb * P:
                                               hb * T + (sc0 + j) * CH
                                               + sb * P + P],
                                            qt[:, CH:2 * CH],
                                            start=(sb == 0), stop=(sb == 1))
                                nc.scalar.activation(ex[:], ps[:], AF.Exp)
                                nc.vector.tensor_mul(
                                    ex[:], ex[:],
                                    masks[:, sc0 * NB:(sc0 + 2) * NB])
                            return ex

                        def emit_av(h, u, ex, av):
                            if u < 4:
                                # qh0 and qh1 columns are adjacent in both ex
                                # and av: one [65, 512] matmul per s-block.
                                for sb in range(2):
                                    vslice = vt[:, (u * 2 + sb) * H * VW
                                                + h * VW:
                                                (u * 2 + sb) * H * VW
                                                + (h + 1) * VW]
                                    nc.tensor.matmul(
                                        av[:, 0:2 * CH], vslice,
                                        ex[:, sb * NB:(sb + 1) * NB],
                                        start=(u == 0 and sb == 0),
                                        stop=(u == 3 and sb == 1),
                                        skip_group_check=True)
                            else:
                                for i in range(4):
                                    sc = 4 + 2 * (u - 4) + i // 2
                                    sb = i % 2
                                    vslice = vt[:, (sc * 2 + sb) * H * VW
                                                + h * VW:
                                                (sc * 2 + sb) * H * VW
                                                + (h + 1) * VW]
                                    nc.tensor.matmul(
                                        av[:, CH:2 * CH], vslice,
                                        ex[:, (i // 2) * NB + sb * CH:
                                           (i // 2) * NB + (sb + 1) * CH],
                                        start=False,
                                        stop=(u == 5 and i == 3),
                                        skip_group_check=True)

                        def emit_tail(h, av):
                            hb, ho = h // 2, (h % 2) * D
                            lnz = att.tile([1, 2 * CH], f32, name="lnz")
                            nc.scalar.activation(lnz[:], av[D:D + 1, :],
                                                 AF.Ln)
                            # negate on the DVE: an Exp with scale=-1 would
                            # reprogram the ACT table and force a reload for
                            # the next unit's plain Exp (2x1.28us per tail).
                            lnzn = att.tile([1, 2 * CH], f32, name="lnzn")
                            nc.vector.tensor_scalar_mul(lnzn[:], lnz[:], -1.0)
                            rzr = att.tile([1, 2 * CH], f32r, name="rzr")
                            nc.scalar.activation(rzr[:], lnzn[:], AF.Exp)
                            bc = pbc2.tile([D, 2 * CH], f32, name="bc")
                            nc.tensor.matmul(bc[:], ones_row[0:1, 0:D],
                                             rzr[:], start=True, stop=True)
                            bcs = att.tile([D, 2 * CH], f32, name="bcs")
                            nc.vector.tensor_copy(bcs[:], bc[:])
                            nc.vector.tensor_mul(
                                aT[ho:ho + D, hb * OWN:(hb + 1) * OWN],
                                av[0:D, :], bcs[:])

                        # Two heads run interleaved through the unit
                        # pipeline (PE always has the sibling head's matmuls
                        # while ACT/DVE chew exp+mask); AV trails by two
                        # pipeline slots; head tails: first at pair end,
                        # second deferred into the next pair.
                        prev = None        # (h, av) awaiting tail
                        for hp in range(0, H, 2):
                            avs = {hp: pav.tile([VW, 2 * CH], f32,
                                                name="av"),
                                   hp + 1: pav.tile([VW, 2 * CH], f32,
                                                    name="av")}
                            pend = []      # (h, u, ex) awaiting AV
                            for u in range(6):
                                for h in (hp, hp + 1):
                                    ex = emit_unit(h, u)
                                    if len(pend) >= 2:
                                        ph, pu, pex = pend.pop(0)
                                        emit_av(ph, pu, pex, avs[ph])
                                    pend.append((h, u, ex))
                                if u == 2 and prev is not None:
                                    emit_tail(*prev)
                                    prev = None
                            for ph, pu, pex in pend:
                                emit_av(ph, pu, pex, avs[ph])
                            emit_tail(hp, avs[hp])
                            prev = (hp + 1, avs[hp + 1])
                        emit_tail(*prev)

                # ---- proj + residual -> x1T (fp32) ----
                with tc.tile_pool(name="wpp", bufs=1) as wpp, \
                     tc.tile_pool(name="evp", bufs=4) as evp, \
                     tc.tile_pool(name="psp2", bufs=4, space="PSUM") as psp2:
                    wproj = wpp.tile([P, KC * C], bf16, name="wproj")
                    nc.sync.dma_start(wproj[:], wproj_in)
                    xTo = wpp.tile([P, KC * OWN], f32r, name="xTo")
                    nc.sync.dma_start(xTo[:], xTown_in)
                    for m in range(KC):
                        ps = psp2.tile([P, NB], f32, name="ps")
                        for kc in range(KC):
                            nc.tensor.matmul(
                                ps[:],
                                wproj[:, m * C + kc * P:m * C + kc * P + P],
                                aT[:, kc * OWN:(kc + 1) * OWN],
                                start=(kc == 0), stop=(kc == KC - 1))
                        nc.vector.scalar_tensor_tensor(
                            x1T[:, m * OWN:(m + 1) * OWN], ps[:],
                            bproj[:, m:m + 1],
                            xTo[:, m * OWN:(m + 1) * OWN],
                            ALU.add, ALU.add)

            # ---- LN2 + MLP ----
            with tc.tile_pool(name="mlpw", bufs=3) as mlpw, \
                 tc.tile_pool(name="ln2p", bufs=1) as ln2p, \
                 tc.tile_pool(name="hp", bufs=1) as hp, \
                 tc.tile_pool(name="lnw2", bufs=3) as lnw2, \
                 tc.tile_pool(name="evp2", bufs=4) as evp2, \
                 tc.tile_pool(name="pstat2", bufs=1, space="PSUM") as pstat2, \
                 tc.tile_pool(name="pbc3", bufs=1, space="PSUM") as pbc3, \
                 tc.tile_pool(name="psp3", bufs=3, space="PSUM") as psp3:
                ln2T = ln2p.tile([P, KC * OWN], bf16, name="ln2T")
                bcB, bcR = ln_stats(
                    x1T, lambda kc: slice(kc * OWN, (kc + 1) * OWN),
                    OWN, lnw2, pstat2, pbc3)
                ln_apply(ln2T, lambda kc: slice(kc * OWN, (kc + 1) * OWN),
                         x1T, lambda kc: slice(kc * OWN, (kc + 1) * OWN),
                         bcB, bcR, OWN, lnw2)

                hT = hp.tile([P, FB * OWN], bf16, name="hT")
                for half in range(2):
                    wl1 = mlpw.tile([P, FB * C // 2], bf16, name="wl1",
                                    tag="w")
                    nc.sync.dma_start(
                        wl1[:], wl1_in[:, half * FB * C // 2:
                                       (half + 1) * FB * C // 2])
                    for mm_ in range(FB // 2):
                        m = half * (FB // 2) + mm_
                        ps = psp3.tile([P, NB], f32, name="ps")
                        for kc in range(KC):
                            nc.tensor.matmul(
                                ps[:],
                                wl1[:, mm_ * C + kc * P:mm_ * C + kc * P + P],
                                ln2T[:, kc * OWN:(kc + 1) * OWN],
                                start=(kc == 0), stop=(kc == KC - 1))
                        nc.scalar.activation(hT[:, m * OWN:(m + 1) * OWN],
                                             ps[:], AF.Relu,
                                             bias=bl1[:, m:m + 1])

                for half in range(2):
                    wl3 = mlpw.tile([P, KC * FF // 2], bf16, name="wl3",
                                    tag="w")
                    nc.sync.dma_start(
                        wl3[:], wl3_in[:, half * KC * FF // 2:
                                       (half + 1) * KC * FF // 2])
                    for mm_ in range(KC // 2):
                        m = half * (KC // 2) + mm_
                        ps = psp3.tile([P, NB], f32, name="ps")
                        for fc in range(FB):
                            nc.tensor.matmul(
                                ps[:],
                                wl3[:, mm_ * FF + fc * P:
                                    mm_ * FF + fc * P + P],
                                hT[:, fc * OWN:(fc + 1) * OWN],
                                start=(fc == 0), stop=(fc == FB - 1))
                        o = evp2.tile([P, NB], f32, name="o")
                        nc.vector.scalar_tensor_tensor(
                            o[:], ps[:], bl3[:, m:m + 1],
                            x1T[:, m * OWN:(m + 1) * OWN],
                            ALU.add, ALU.add)
                        nc.sync.dma_start(outT[m * P:(m + 1) * P, :], o[:])

        if n_iters == 1:
            with ExitStack() as ctx2:
                body(ctx2)
        else:
            with tc.For_i(0, n_iters, 1):
                with ExitStack() as ctx2:
                    body(ctx2)

    return nc


# The host permutes each core's token order so its own zigzag chunks
# always sit at chunk positions 0 (lo) and 7 (hi); Q then reads ln1T at
# fixed columns, and masks are host-built in the permuted coordinates.
OWN_CHUNK_OFF = (0, 7)


def _pack_lhsT(w):
    """[M, K] weight (out = w @ x) -> [128, (M/128)*(K/128)*128] with
    [p, m*K + kc*128 + j] = w[m*128 + j, kc*128 + p]."""
    M, K = w.shape
    nm, nk = M // P, K // P
    return np.ascontiguousarray(
        w.reshape(nm, P, nk, P).transpose(3, 0, 2, 1).reshape(P, nm * K))


def _host_prep(x, qkv_w, proj_w, proj_b, l1_w, l1_b, l3_w, l3_b,
               ln1_g, ln1_b, ln2_g, ln2_b):
    import ml_dtypes
    f = np.float32
    bf = ml_dtypes.bfloat16
    x = np.asarray(x, f)
    qkv_w = np.asarray(qkv_w, f)
    scale = np.float32(D ** -0.5)
    w_eff = qkv_w * np.asarray(ln1_g, f)[None, :]
    b_eff = (qkv_w @ np.asarray(ln1_b, f)).astype(f)
    w_eff[:C] *= scale
    b_eff[:C] *= scale
    l1_eff = np.asarray(l1_w, f) * np.asarray(ln2_g, f)[None, :]
    bl1_eff = (np.asarray(l1_b, f)
               + np.asarray(l1_w, f) @ np.asarray(ln2_b, f)).astype(f)

    # head-pair biases [128, m]: [p, m] = b[m*128 + p]
    def colpack(b):
        return np.asarray(b, f).reshape(-1, P).T  # [128, nm]

    bias_all = np.zeros((P, 64), f)
    bias_all[:, 0:8] = colpack(b_eff[0:C])
    bias_all[:, 8:16] = colpack(b_eff[C:2 * C])
    bias_all[:, 16:24] = colpack(np.asarray(proj_b, f))
    bias_all[:, 24:56] = colpack(bl1_eff)
    bias_all[:, 56:64] = colpack(np.asarray(l3_b, f))

    shared = {
        "wqk_in": _pack_lhsT(w_eff[0:2 * C]).astype(bf),
        # V rhs pack: [p, kc*C + vcol] = w_eff[2C + vcol, kc*128 + p]
        "wv_in": np.ascontiguousarray(
            w_eff[2 * C:3 * C].reshape(C, KC, P).transpose(2, 1, 0)
            .reshape(P, KC * C)).astype(bf),
        "wproj_in": _pack_lhsT(np.asarray(proj_w, f)).astype(bf),
        "wl1_in": _pack_lhsT(l1_eff).astype(bf),
        "wl3_in": _pack_lhsT(np.asarray(l3_w, f)).astype(bf),
        "bias_in": bias_all,
        "bv_in": np.ascontiguousarray(
            np.broadcast_to(b_eff[2 * C:3 * C][None, :], (P, C))),
    }

    in_maps = []
    for cid in range(N_CORES):
        b, r = divmod(cid, RANKS)
        lo, hi = r, NCHUNK - 1 - r
        # permute token chunks so own chunks sit at positions 0 and 7:
        # perm[newchunk] = oldchunk; own lo -> 0, own hi -> 7, others keep
        # relative order in between.
        rest = [c for c in range(NCHUNK) if c not in (lo, hi)]
        perm = [lo] + rest + [hi]
        old_order = np.asarray(perm)
        tok_perm = (old_order[:, None] * CH
                    + np.arange(CH)[None, :]).reshape(-1)  # new idx -> old tok
        xb = x[b][tok_perm]                                # [T, C] permuted
        # xT layout [p, kc*T + t] = xb[t, kc*128 + p]
        xT_in = np.ascontiguousarray(
            xb.T.reshape(KC, P, T).transpose(1, 0, 2).reshape(P, KC * T)
        ).astype(bf)
        x_own = xb[np.r_[0:CH, (NCHUNK - 1) * CH:T]]       # chunks 0,7 = own
        xTown_in = np.ascontiguousarray(
            x_own.T.reshape(KC, P, OWN).transpose(1, 0, 2).reshape(P, KC * OWN))

        # masks in PERMUTED coordinates: new chunk nc_ holds old chunk
        # perm[nc_]; query chunks: qh0 = new 0 (old lo), qh1 = new 7 (old hi).
        # causal: old_s_token <= old_q_token.
        m2 = np.zeros((NCHUNK, P, 2, CH), f)
        for nsc in range(NCHUNK):
            osc = perm[nsc]
            for sb in range(2):
                s_old = osc * CH + sb * P + np.arange(P)  # [128]
                if nsc < 4:
                    q_old = lo * CH + np.arange(CH)
                else:
                    q_old = hi * CH + np.arange(CH)
                m2[nsc, :, sb, :] = (s_old[:, None] <= q_old[None, :])
        mask_in = np.ascontiguousarray(
            m2.transpose(1, 0, 2, 3).reshape(P, NCHUNK * NB)).astype(bf)

        in_maps.append({
            "xT_in": xT_in,
            "xTown_in": xTown_in,
            "mask_in": mask_in,
            **shared,
        })
    return in_maps


def _assemble(results):
    out = np.empty((B, T, C), np.float32)
    for cid in range(N_CORES):
        b, r = divmod(cid, RANKS)
        lo, hi = r, NCHUNK - 1 - r
        oT = results[cid]["outT"]
        out[b, lo * CH:(lo + 1) * CH] = oT[:, 0:CH].T
        out[b, hi * CH:(hi + 1) * CH] = oT[:, CH:2 * CH].T
    return out


_CACHE = {}


def get_nc(n_iters=1):
    if n_iters not in _CACHE:
        import concourse.bacc as bacc
        import concourse.tile as tile
        from concourse import mybir
        nc = bacc.Bacc("TRN2", target_bir_lowering=False, debug=False,
                       num_devices=N_CORES)
        build_core_program(nc, tile, mybir, n_iters=n_iters)
        nc.compile()
        _CACHE[n_iters] = nc
    return _CACHE[n_iters]


def run(inputs, n_iters=1):
    from concourse.bass_utils import run_bass_kernel_spmd
    in_maps = _host_prep(**inputs)
    nc = get_nc(n_iters)
    res = run_bass_kernel_spmd(nc, in_maps, list(range(N_CORES)))
    return _assemble(res.results)


def kernel(**inputs):
    return run(inputs, n_iters=1)

